# revision 1
# baseline (speedup 1.0000x reference)
"""Trainium2 Bass kernel for nn_CausalSelfAttention_87978110091517.

Causal self-attention with data-dependent per-token RoPE, fork-plan channel
overwrites, padding mask, and exp(cumulative_scores) value scaling.

Sharding: 8 cores = 2 (batch) x 4 (head groups of 4 heads).  Each core
computes qkv projection for its 4 heads, RoPE, causal attention, and a
partial output projection; the host sums the 4 partials per batch.

Shapes (hardcoded): B=2, T=2048, C=1024, H=16, HD=64, BLOCK=2048.
"""

import sys

for _p in ("/opt/trn_rl_repo", "/root/.axon_site/_ro/trn_rl_repo"):
    if _p not in sys.path:
        sys.path.insert(0, _p)

import numpy as np

import concourse.bacc as bacc
import concourse.tile as tile
import concourse.mybir as mybir
from concourse.bass import ts
from concourse.bass_utils import run_bass_kernel_spmd
from concourse.masks import make_identity

F16 = mybir.dt.float16
F32 = mybir.dt.float32
F32R = mybir.dt.float32r
F8 = mybir.dt.float8e4

B, T, C = 2, 2048, 1024
H, HD = 16, 64
BLOCK = 2048
ROPE_BASE = 10000.0
NEG_BIG = -1.0e30
N_CORES = 8
HG = H // (N_CORES // B)  # heads per core = 4
CL = HG * HD              # local channels = 256
P = 128
NT = T // P               # 16 t-tiles
NW = 4                    # q windows of 512
WQ = 512

_PROGRAM_CACHE = {}


def _build_program(with_qkv_bias: bool, repeat: int = 1, phases: str = 'all',
                   use_kbias: bool = False):
    """Build the per-core Bass program (identical across cores)."""
    nc = bacc.Bacc("TRN2", target_bir_lowering=False, debug=False)

    xT_t = nc.dram_tensor("xT", [C, T], F16, kind="ExternalInput").ap()
    wl_t = nc.dram_tensor("wl", [C, 3 * CL], F16, kind="ExternalInput").ap()
    w2_t = nc.dram_tensor("w2", [CL, C], F16, kind="ExternalInput").ap()
    ab_t = nc.dram_tensor("ab", [1, 3 * CL], F16, kind="ExternalInput").ap()
    cosT_t = nc.dram_tensor("cosT", [P, T], F16, kind="ExternalInput").ap()
    sinsT_t = nc.dram_tensor("sinsT", [P, T], F16, kind="ExternalInput").ap()
    ones16_t = nc.dram_tensor("ones16", [T], F16, kind="ExternalInput").ap()
    csf16_t = nc.dram_tensor("csf16", [T], F16, kind="ExternalInput").ap()
    ecs_t = nc.dram_tensor("ecs", [T], F32, kind="ExternalInput").ap()
    kb_t = nc.dram_tensor("kb", [T], F32, kind="ExternalInput").ap()
    out_t = nc.dram_tensor("out", [T, C], F32, kind="ExternalOutput").ap()

    from contextlib import ExitStack

    with ExitStack() as top:
        tc = top.enter_context(tile.TileContext(nc))

        const = top.enter_context(tc.tile_pool(name="const", bufs=1))
        persist = top.enter_context(tc.tile_pool(name="persist", bufs=1))

        ident = const.tile([P, P], F16)
        make_identity(nc, ident[:])
        if with_qkv_bias:
            ones1 = const.tile([1, P], F16)
            nc.vector.memset(ones1[:], 1.0)
            ones512 = const.tile([1, WQ], F16)
            nc.vector.memset(ones512[:], 1.0)
            absb = const.tile([1, 3 * CL], F16)
            nc.sync.dma_start(absb[:], ab_t[:])
        import concourse.bass as bass
        ones16_bc = bass.AP(tensor=ones16_t.tensor, offset=0,
                            ap=[[0, 2], [1, T]])
        csf16_bc = bass.AP(tensor=csf16_t.tensor, offset=0,
                           ap=[[0, 2], [1, T]])

        w2sb = persist.tile([P, 2, C], F16)
        ecs_sb = persist.tile([P, NT], F32)
        kb_sb = persist.tile([P, NT], F32)

        qT = persist.tile([P, 2, T], F16)   # [2*64 head-dims, chunk, t]
        kT = persist.tile([P, 2, T], F16)
        vaug = persist.tile([P, NT, HG, HD + 1], F16)
        yns = persist.tile([P, NT, CL], F16)
        yT = persist.tile([P, 2, T], F16)

        for _rep in range(repeat):
            with ExitStack() as ph:
                big = ph.enter_context(tc.tile_pool(name="big", bufs=1))
                rp = ph.enter_context(tc.tile_pool(name="rope", bufs=3))
                expool = ph.enter_context(tc.tile_pool(name="expool", bufs=4))
                tmp = ph.enter_context(tc.tile_pool(name="tmp", bufs=4))
                psA = ph.enter_context(tc.tile_pool(name="psA", bufs=2, space="PSUM"))
                psC = ph.enter_context(tc.tile_pool(name="psC", bufs=3, space="PSUM"))
                psD = ph.enter_context(tc.tile_pool(name="psD", bufs=2, space="PSUM"))
                psF = ph.enter_context(tc.tile_pool(name="psF", bufs=1, space="PSUM"))

                wsb = big.tile([P, 8, 3 * CL], F16)
                wl_r = wl_t.rearrange("(c p) w -> p c w", p=P)
                for c in range(8):
                    nc.sync.dma_start(wsb[:, c, 0 : 2 * CL], wl_r[:, c, 0 : 2 * CL])
                xsb = big.tile([P, 8, T], F16)
                xT_r = xT_t.rearrange("(c p) t -> p c t", p=P)
                cosTsb = big.tile([P, T], F16)
                sinsTsb = big.tile([P, T], F16)
                nc.sync.dma_start(xsb[:, :, 0:512], xT_r[:, :, 0:512])
                nc.sync.dma_start(cosTsb[:], cosT_t[:])
                nc.sync.dma_start(sinsTsb[:], sinsT_t[:])
                for c in range(8):
                    nc.sync.dma_start(wsb[:, c, 2 * CL : 3 * CL],
                                      wl_r[:, c, 2 * CL : 3 * CL])
                nc.sync.dma_start(ecs_sb[:], ecs_t.rearrange("(n p) -> p n", p=P))
                nc.sync.dma_start(kb_sb[:], kb_t.rearrange("(n p) -> p n", p=P))
                for quarter in range(1, 4):
                    sl = slice(quarter * 512, (quarter + 1) * 512)
                    nc.sync.dma_start(xsb[:, :, sl], xT_r[:, :, sl])
                nc.sync.dma_start(w2sb[:], w2_t.rearrange("(c p) o -> p c o", p=P))

                # q/k produced directly in transposed [head-dim, t] layout.
                # Host permutes the head dim so RoPE pairs (d, d+32) share a
                # 32-partition quadrant: rotate_half = stream_shuffle swap,
                # sign baked into sinsT.
                SWAP16 = list(range(16, 32)) + list(range(16))

                def aunit(tw):
                    """qkT + v production for t-window tw (512 tokens)."""
                    for wg in range(4):
                        dst = qT if wg < 2 else kT
                        g = wg % 2
                        pp = psA.tile([P, WQ], F32, tag="pp")
                        last = 7 if not with_qkv_bias else -1
                        for c in range(8):
                            nc.tensor.matmul(
                                pp[:], wsb[:, c, wg * P : (wg + 1) * P],
                                xsb[:, c, ts(tw, WQ)],
                                start=(c == 0), stop=(c == last))
                        if with_qkv_bias:
                            nc.tensor.matmul(
                                pp[:], absb[0:1, wg * P : (wg + 1) * P],
                                ones512[0:1, :], start=False, stop=True)
                        raw = rp.tile([P, WQ], F16, tag="raw")
                        nc.vector.tensor_copy(raw[:], pp[:])
                        rot = rp.tile([P, WQ], F16, tag="rot")
                        nc.vector.stream_shuffle(rot[:], raw[:], mask=SWAP16)
                        nc.gpsimd.tensor_mul(raw[:], raw[:], cosTsb[:, ts(tw, WQ)])
                        nc.vector.tensor_mul(rot[:], rot[:], sinsTsb[:, ts(tw, WQ)])
                        nc.vector.tensor_add(dst[:, g, ts(tw, WQ)], raw[:], rot[:])
                    for il in range(4):
                        i = 4 * tw + il
                        v_ps = psA.tile([P, CL], F32, tag="pp")
                        last = 7 if not with_qkv_bias else -1
                        for c in range(8):
                            nc.tensor.matmul(
                                v_ps[:], xsb[:, c, ts(i, P)],
                                wsb[:, c, 2 * CL : 3 * CL],
                                start=(c == 0), stop=(c == last))
                        if with_qkv_bias:
                            nc.tensor.matmul(v_ps[:], ones1[:],
                                             absb[0:1, 2 * CL : 3 * CL],
                                             start=False, stop=True)
                        nc.vector.tensor_scalar_mul(
                            vaug[:, i, :, 0:HD],
                            v_ps[:].rearrange("p (hh d) -> p hh d", d=HD),
                            ecs_sb[:, i : i + 1])
                        nc.vector.memset(vaug[:, i, :, HD], 1.0)
                    # fork-plan overwrites for this window: q[...,-1]=1,
                    # k[...,-1]=cumulative_scores (f16), via row DMA
                    for g in range(2):
                        qrow = qT[:, g, ts(tw, WQ)].rearrange(
                            "(a b) t -> a b t", b=HD)[:, HD - 1, :]
                        nc.sync.dma_start(qrow, ones16_bc[:, ts(tw, WQ)])
                        krow = kT[:, g, ts(tw, WQ)].rearrange(
                            "(a b) t -> a b t", b=HD)[:, HD - 1, :]
                        nc.sync.dma_start(krow, csf16_bc[:, ts(tw, WQ)])

                def scores_chunk(w, h, ex, kc):
                    """One k-chunk of scores + exp (+ causal mask) for (w,h)."""
                    h2, hg = h % 2, h // 2
                    hlo, hhi = HD * h2, HD * h2 + HD
                    d = kc - 4 * w
                    width = WQ if d < 0 else WQ - P * d
                    q0 = WQ * w + (WQ - width)
                    sc = psC.tile([P, WQ], F32, tag="sc")
                    nc.tensor.matmul(
                        sc[:, 0:width],
                        kT[hlo:hhi, hg, ts(kc, P)],
                        qT[hlo:hhi, hg, q0 : q0 + width],
                        start=True, stop=True)
                    if use_kbias:
                        nc.scalar.activation(
                            ex[:, kc, 0:width], sc[:, 0:width],
                            mybir.ActivationFunctionType.Exp,
                            bias=kb_sb[:, kc : kc + 1], scale=0.125)
                    else:
                        nc.scalar.activation(
                            ex[:, kc, 0:width], sc[:, 0:width],
                            mybir.ActivationFunctionType.Exp,
                            scale=0.125)
                    if d >= 0:
                        nc.gpsimd.affine_select(
                            ex[:, kc, 0:P], ex[:, kc, 0:P],
                            pattern=[[1, P]],
                            compare_op=mybir.AluOpType.is_ge,
                            fill=0.0, base=0, channel_multiplier=-1)

                def attn_qt(w, h, ex, ql):
                    """attn @ v + normalization for one q-subtile of (w,h)."""
                    qt = 4 * w + ql
                    y_ps = psD.tile([P, HD + 1], F32, tag="y_ps")
                    for kc in range(qt + 1):
                        d = kc - 4 * w
                        toff = P * (qt - 4 * w) - (0 if d < 0 else P * d)
                        nc.tensor.matmul(
                            y_ps[:], ex[:, kc, toff : toff + P],
                            vaug[:, kc, h, :],
                            start=(kc == 0), stop=(kc == qt))
                    rc = tmp.tile([P, 1], F32, tag="rc")
                    nc.vector.reciprocal(rc[:], y_ps[:, HD : HD + 1])
                    nc.vector.tensor_scalar_mul(
                        yns[:, qt, ts(h, HD)], y_ps[:, 0:HD], rc[:])

                def epi_qt(w, ql):
                    """y transpose + output projection for one q-subtile."""
                    qt = 4 * w + ql
                    for g in range(2):
                        ty = psF.tile([P, P], F16, tag="oty")
                        nc.tensor.transpose(ty[:], yns[:, qt, ts(g, P)],
                                            ident[:])
                        nc.vector.tensor_copy(yT[:, g, ts(qt, P)], ty[:])
                    for half in range(2):
                        o_ps = psF.tile([P, 512], F32, tag="oty")
                        for cg in range(2):
                            nc.tensor.matmul(
                                o_ps[:], yT[:, cg, ts(qt, P)],
                                w2sb[:, cg, ts(half, 512)],
                                start=(cg == 0), stop=(cg == 1))
                        o_sb = tmp.tile([P, 512], F32, tag="o_sb")
                        nc.vector.tensor_copy(o_sb[:], o_ps[:])
                        nc.sync.dma_start(out_t[ts(qt, P), ts(half, 512)],
                                          o_sb[:])

                # Unified software pipeline, chunk-granular: while a
                # stream's exps drain on ACT, the PE queue carries the next
                # stream's scores matmuls; attn/epilogue work-slices of
                # previous streams are spread between scores chunks.
                if phases == 'a':
                    for tw in range(4):
                        aunit(tw)
                    continue
                from collections import deque
                slices = deque()

                def drain(n):
                    for _ in range(n):
                        if slices:
                            slices.popleft()()

                aunit(0)
                for w in range(NW):
                    nkc = 4 * w + 4
                    for h in range(HG):
                        ex = expool.tile([P, NT, WQ], F16, tag="ex",
                                         name=f"ex_{w}_{h}")
                        per = max(1, (nkc + 3) // 4)
                        for kc in range(nkc):
                            scores_chunk(w, h, ex, kc)
                            if kc % per == per - 1:
                                drain(1)
                        for ql in range(4):
                            slices.append(
                                lambda w=w, h=h, ex=ex, ql=ql:
                                attn_qt(w, h, ex, ql))
                        if h == HG - 1:
                            if w < NW - 1:
                                aunit(w + 1)
                            for ql in range(4):
                                slices.append(
                                    lambda w=w, ql=ql: epi_qt(w, ql))
                drain(len(slices))

    nc.compile()
    return nc


def _get_program(with_qkv_bias: bool, repeat: int = 1, phases: str = 'all',
                 use_kbias: bool = False):
    key = (bool(with_qkv_bias), repeat, phases, bool(use_kbias))
    if key not in _PROGRAM_CACHE:
        _PROGRAM_CACHE[key] = _build_program(*key)
    return _PROGRAM_CACHE[key]


def _host_prep(x, cumulative_scores, padding_mask, token_index,
               c_attn_weight, c_attn_bias, c_proj_weight):
    """Host-side input prep: sharding + small index/trig math (O(B*T*HD))."""
    x = np.asarray(x, np.float32)
    cs = np.asarray(cumulative_scores, np.float32)
    pm = np.asarray(padding_mask, bool)
    ti = np.asarray(token_index, np.int64)
    Wa = np.asarray(c_attn_weight, np.float32)
    ba = np.asarray(c_attn_bias, np.float32)
    Wp = np.asarray(c_proj_weight, np.float32)

    # token histogram accumulates over ALL batches (faithful to reference)
    cnt = np.bincount(ti.ravel(), minlength=BLOCK).astype(np.float32)
    inv = np.float32(1.0) / (cnt + np.float32(1e-10))
    gathered = np.take_along_axis(np.broadcast_to(inv, (B, BLOCK)), ti, axis=1)
    pr = np.cumsum(gathered.astype(np.float32), axis=1, dtype=np.float32)

    inv_freq = 1.0 / ROPE_BASE ** (np.arange(0, HD, 2, dtype=np.float64) / HD)

    # permutation putting RoPE pair partners (d, d+32) in the same
    # 32-partition quadrant: [0:16, 32:48, 16:32, 48:64]
    perm = np.r_[0:16, 32:48, 16:32, 48:64]
    # cosT/sinsT tables in [head-dim(permuted) x 2 heads, t] layout; the
    # rotate_half sign is baked into sinsT.
    d_of_p = np.concatenate([perm, perm])          # [128]
    j_of_p = d_of_p % (HD // 2)
    sign_of_p = np.where(d_of_p < HD // 2, -1.0, 1.0)
    ang = pr[:, None, :].astype(np.float64) * inv_freq[j_of_p][None, :, None]
    cosT = np.cos(ang).astype(np.float16)                       # [B,128,T]
    sinsT = (sign_of_p[None, :, None] * np.sin(ang)).astype(np.float16)

    pmg = np.take_along_axis(pm, ti, axis=1)  # [B,T] gathered padding mask
    kb = np.where(pmg, np.float32(0.0), np.float32(NEG_BIG))
    ecs = np.exp(cs).astype(np.float32)
    csf16 = cs.astype(np.float16)
    ones16 = np.ones(T, np.float16)

    W3 = Wa.reshape(C, 3, H, HD)
    b3 = ba.reshape(3, H, HD)

    in_maps = []
    for core in range(N_CORES):
        b = core // (N_CORES // B)
        g0 = (core % (N_CORES // B)) * HG
        wq = W3[:, 0, g0 : g0 + HG, :][:, :, perm].reshape(C, CL)
        wk = W3[:, 1, g0 : g0 + HG, :][:, :, perm].reshape(C, CL)
        wv = W3[:, 2, g0 : g0 + HG, :].reshape(C, CL)
        wl = np.concatenate([wq, wk, wv], axis=1)
        ab = np.concatenate(
            [b3[0, g0 : g0 + HG, :][:, perm].reshape(CL),
             b3[1, g0 : g0 + HG, :][:, perm].reshape(CL),
             b3[2, g0 : g0 + HG, :].reshape(CL)])[None, :]
        w2 = Wp[g0 * HD : (g0 + HG) * HD, :].astype(np.float16)
        in_maps.append({
            "xT": np.ascontiguousarray(x[b].T).astype(np.float16),
            "wl": np.ascontiguousarray(wl).astype(np.float16),
            "w2": w2,
            "ab": np.ascontiguousarray(ab, np.float16),
            "cosT": cosT[b],
            "sinsT": sinsT[b],
            "ones16": ones16,
            "csf16": csf16[b],
            "ecs": ecs[b],
            "kb": kb[b],
        })
    return in_maps


def kernel(x, cumulative_scores, padding_mask, token_index,
           c_attn_weight, c_attn_bias, c_proj_weight, c_proj_bias):
    in_maps = _host_prep(x, cumulative_scores, padding_mask, token_index,
                         c_attn_weight, c_attn_bias, c_proj_weight)
    with_bias = bool(np.any(np.asarray(c_attn_bias) != 0))
    use_kbias = any((m["kb"] != 0).any() for m in in_maps)
    nc = _get_program(with_bias, 1, 'all', use_kbias)
    res = run_bass_kernel_spmd(nc, in_maps, list(range(N_CORES)))
    ncb = N_CORES // B
    bp = np.asarray(c_proj_bias, np.float32)
    out = np.empty((B, T, C), np.float32)
    for b in range(B):
        acc = res.results[b * ncb]["out"].astype(np.float32)
        for j in range(1, ncb):
            acc = acc + res.results[b * ncb + j]["out"]
        out[b] = acc + bp[None, :]
    return out



# revision 17
# speedup vs baseline: 1.2569x; 1.2569x over previous
"""Trainium2 Bass kernel for nn_CausalSelfAttention_87978110091517.

Causal self-attention with data-dependent per-token RoPE, fork-plan channel
overwrites, padding mask, and exp(cumulative_scores) value scaling.

Sharding: 8 cores = 2 (batch) x 4 (head groups of 4 heads).  Each core
computes qkv projection for its 4 heads, RoPE, causal attention, and a
partial output projection; the host sums the 4 partials per batch.

Shapes (hardcoded): B=2, T=2048, C=1024, H=16, HD=64, BLOCK=2048.
"""

import sys

for _p in ("/opt/trn_rl_repo", "/root/.axon_site/_ro/trn_rl_repo"):
    if _p not in sys.path:
        sys.path.insert(0, _p)

import numpy as np

import concourse.bacc as bacc
import concourse.tile as tile
import concourse.mybir as mybir
from concourse.bass import ts
from concourse.bass_utils import run_bass_kernel_spmd
from concourse.masks import make_identity

F16 = mybir.dt.float16
F32 = mybir.dt.float32
F32R = mybir.dt.float32r
F8 = mybir.dt.float8e4

B, T, C = 2, 2048, 1024
H, HD = 16, 64
BLOCK = 2048
ROPE_BASE = 10000.0
NEG_BIG = -1.0e30
N_CORES = 8
HG = H // (N_CORES // B)  # heads per core = 4
CL = HG * HD              # local channels = 256
P = 128
NT = T // P               # 16 t-tiles
NW = 4                    # q windows of 512
WQ = 512

_PROGRAM_CACHE = {}


def _build_program(with_qkv_bias: bool, repeat: int = 1, phases: str = 'all',
                   use_kbias: bool = False, dup: str = ''):
    """Build the per-core Bass program (identical across cores).

    dup: timing-probe flags — 'p' doubles every PE matmul, 'a' doubles
    every ACT activation, 'v' doubles key DVE ops. Numerics are garbage
    with any flag set; used only to locate the bottleneck engine.
    """
    nc = bacc.Bacc("TRN2", target_bir_lowering=False, debug=False)
    DUP_PE = 'p' in dup
    DUP_ACT = 'a' in dup
    DUP_DVE = 'v' in dup

    xT_t = nc.dram_tensor("xT", [C, T], F16, kind="ExternalInput").ap()
    wl_t = nc.dram_tensor("wl", [C, 3 * CL], F16, kind="ExternalInput").ap()
    w2_t = nc.dram_tensor("w2", [CL, C], F16, kind="ExternalInput").ap()
    ab_t = nc.dram_tensor("ab", [1, 3 * CL], F16, kind="ExternalInput").ap()
    cosT_t = nc.dram_tensor("cosT", [P, T], F16, kind="ExternalInput").ap()
    sinsT_t = nc.dram_tensor("sinsT", [P, T], F16, kind="ExternalInput").ap()
    ones16_t = nc.dram_tensor("ones16", [T], F16, kind="ExternalInput").ap()
    csf16_t = nc.dram_tensor("csf16", [T], F16, kind="ExternalInput").ap()
    ecs_t = nc.dram_tensor("ecs", [T], F32, kind="ExternalInput").ap()
    kb_t = nc.dram_tensor("kb", [T], F32, kind="ExternalInput").ap()
    out_t = nc.dram_tensor("out", [T, C], F32, kind="ExternalOutput").ap()

    from contextlib import ExitStack

    with ExitStack() as top:
        tc = top.enter_context(tile.TileContext(nc))

        const = top.enter_context(tc.tile_pool(name="const", bufs=1))
        persist = top.enter_context(tc.tile_pool(name="persist", bufs=1))

        ident = const.tile([P, P], F16)
        make_identity(nc, ident[:])
        if with_qkv_bias:
            ones1 = const.tile([1, P], F16)
            nc.vector.memset(ones1[:], 1.0)
            ones512 = const.tile([1, WQ], F16)
            nc.vector.memset(ones512[:], 1.0)
            absb = const.tile([1, 3 * CL], F16)
            nc.sync.dma_start(absb[:], ab_t[:])
        import concourse.bass as bass
        ones16_bc = bass.AP(tensor=ones16_t.tensor, offset=0,
                            ap=[[0, 2], [1, T]])
        csf16_1p = bass.AP(tensor=csf16_t.tensor, offset=0,
                           ap=[[0, 1], [1, T]])

        w2sb = persist.tile([P, 2, C], F16)
        ecs_sb = persist.tile([P, NT], F32)
        kb_sb = persist.tile([P, NT], F32)

        qT = persist.tile([P, 2, T], F16)   # [2*64 head-dims, chunk, t]
        # k stored zero-padded per head: slot h holds head h's 64 dims in
        # its own partition half, zeros elsewhere, so the scores matmul
        # contracts K=128 (fast PE path) instead of K=64.
        kTp = persist.tile([P, HG, T], F16)
        nc.vector.memset(kTp[HD:P, 0, :], 0.0)
        nc.vector.memset(kTp[HD:P, 2, :], 0.0)
        nc.vector.memset(kTp[0:HD, 1, :], 0.0)
        nc.vector.memset(kTp[0:HD, 3, :], 0.0)
        vaug = persist.tile([P, NT, HG, HD + 1], F16)
        yns = persist.tile([P, NT, CL], F16)
        yT = persist.tile([P, 2, T], F16)

        for _rep in range(repeat):
            with ExitStack() as ph:
                big = ph.enter_context(tc.tile_pool(name="big", bufs=1))
                rp = ph.enter_context(tc.tile_pool(name="rope", bufs=3))
                expool = ph.enter_context(tc.tile_pool(name="expool", bufs=4))
                tmp = ph.enter_context(tc.tile_pool(name="tmp", bufs=4))
                psA = ph.enter_context(tc.tile_pool(name="psA", bufs=2, space="PSUM"))
                psC = ph.enter_context(tc.tile_pool(name="psC", bufs=3, space="PSUM"))
                psD = ph.enter_context(tc.tile_pool(name="psD", bufs=2, space="PSUM"))
                psF = ph.enter_context(tc.tile_pool(name="psF", bufs=1, space="PSUM"))

                wsb = big.tile([P, 8, 3 * CL], F16)
                wl_r = wl_t.rearrange("(c p) w -> p c w", p=P)
                for c in range(8):
                    nc.sync.dma_start(wsb[:, c, 0 : 2 * CL], wl_r[:, c, 0 : 2 * CL])
                xsb = big.tile([P, 8, T], F16)
                xT_r = xT_t.rearrange("(c p) t -> p c t", p=P)
                cosTsb = big.tile([P, T], F16)
                sinsTsb = big.tile([P, T], F16)
                nc.sync.dma_start(xsb[:, :, 0:512], xT_r[:, :, 0:512])
                nc.sync.dma_start(cosTsb[:], cosT_t[:])
                nc.sync.dma_start(sinsTsb[:], sinsT_t[:])
                for c in range(8):
                    nc.sync.dma_start(wsb[:, c, 2 * CL : 3 * CL],
                                      wl_r[:, c, 2 * CL : 3 * CL])
                nc.sync.dma_start(ecs_sb[:], ecs_t.rearrange("(n p) -> p n", p=P))
                nc.sync.dma_start(kb_sb[:], kb_t.rearrange("(n p) -> p n", p=P))
                for quarter in range(1, 4):
                    sl = slice(quarter * 512, (quarter + 1) * 512)
                    nc.sync.dma_start(xsb[:, :, sl], xT_r[:, :, sl])
                nc.sync.dma_start(w2sb[:], w2_t.rearrange("(c p) o -> p c o", p=P))

                # q/k produced directly in transposed [head-dim, t] layout.
                # Host permutes the head dim so RoPE pairs (d, d+32) share a
                # 32-partition quadrant: rotate_half = stream_shuffle swap,
                # sign baked into sinsT.
                SWAP16 = list(range(16, 32)) + list(range(16))

                if dup:
                    burn = ph.enter_context(tc.tile_pool(name="burn", bufs=4))
                    burn_ps = ph.enter_context(
                        tc.tile_pool(name="burn_ps", bufs=2, space="PSUM"))
                    burn_w = big.tile([P, P], F16)
                    nc.vector.memset(burn_w[:], 0.25)
                    burn_src = big.tile([P, WQ], F16)
                    nc.vector.memset(burn_src[:], 0.5)

                def mm(out, l, r, start, stop, **kw):
                    nc.tensor.matmul(out, l, r, start=start, stop=stop, **kw)
                    if DUP_PE:
                        K = l.partition_size()
                        M = l.free_size()
                        N = min(r.free_size(), WQ)
                        bp = burn_ps.tile([P, WQ], F32, tag="bp")
                        nc.tensor.matmul(
                            bp[0:M, 0:N], burn_w[0:K, 0:M],
                            burn_src[0:K, 0:N], start=True, stop=True)

                def act(out, in_, func, **kw):
                    nc.scalar.activation(out, in_, func, **kw)
                    if DUP_ACT:
                        P_ = in_.partition_size()
                        N = in_.free_size()
                        bs = burn.tile([P, WQ], F16, tag="ba")
                        nc.scalar.activation(
                            bs[0:P_, 0:N], burn_src[0:P_, 0:N], func,
                            scale=0.125)

                def dve(op, *a, **kw):
                    op(*a, **kw)
                    if DUP_DVE:
                        in_ = a[1]
                        P_ = in_.partition_size()
                        N = min(in_.free_size(), WQ)
                        bs = burn.tile([P, WQ], F16, tag="bv")
                        nc.vector.tensor_copy(bs[0:P_, 0:N],
                                              burn_src[0:P_, 0:N])

                def aunit(tw):
                    """qkT + v production for t-window tw (512 tokens)."""
                    for wg in range(4):
                        is_q = wg < 2
                        g = wg % 2
                        pp = psA.tile([P, WQ], F32, tag="pp")
                        last = 7 if not with_qkv_bias else -1
                        for c in range(8):
                            mm(pp[:], wsb[:, c, wg * P : (wg + 1) * P],
                               xsb[:, c, ts(tw, WQ)],
                               start=(c == 0), stop=(c == last))
                        if with_qkv_bias:
                            nc.tensor.matmul(
                                pp[:], absb[0:1, wg * P : (wg + 1) * P],
                                ones512[0:1, :], start=False, stop=True)
                        raw = rp.tile([P, WQ], F16, tag="raw")
                        dve(nc.vector.tensor_copy, raw[:], pp[:])
                        rot = rp.tile([P, WQ], F16, tag="rot")
                        dve(nc.vector.stream_shuffle, rot[:], raw[:],
                            mask=SWAP16)
                        nc.gpsimd.tensor_mul(raw[:], raw[:], cosTsb[:, ts(tw, WQ)])
                        dve(nc.vector.tensor_mul, rot[:], rot[:],
                            sinsTsb[:, ts(tw, WQ)])
                        if is_q:
                            dve(nc.vector.tensor_add, qT[:, g, ts(tw, WQ)],
                                raw[:], rot[:])
                        else:
                            dve(nc.vector.tensor_add,
                                kTp[0:HD, 2 * g, ts(tw, WQ)],
                                raw[0:HD, :], rot[0:HD, :])
                            dve(nc.vector.tensor_add,
                                kTp[HD:P, 2 * g + 1, ts(tw, WQ)],
                                raw[HD:P, :], rot[HD:P, :])
                    for il in range(4):
                        i = 4 * tw + il
                        v_ps = psA.tile([P, CL], F32, tag="pp")
                        last = 7 if not with_qkv_bias else -1
                        for c in range(8):
                            mm(v_ps[:], xsb[:, c, ts(i, P)],
                               wsb[:, c, 2 * CL : 3 * CL],
                               start=(c == 0), stop=(c == last))
                        if with_qkv_bias:
                            nc.tensor.matmul(v_ps[:], ones1[:],
                                             absb[0:1, 2 * CL : 3 * CL],
                                             start=False, stop=True)
                        dve(nc.vector.tensor_scalar_mul,
                            vaug[:, i, :, 0:HD],
                            v_ps[:].rearrange("p (hh d) -> p hh d", d=HD),
                            ecs_sb[:, i : i + 1])
                        nc.vector.memset(vaug[:, i, :, HD], 1.0)
                    # fork-plan overwrites for this window: q[...,-1]=1,
                    # k[...,-1]=cumulative_scores (f16), via row DMA
                    for g in range(2):
                        qrow = qT[:, g, ts(tw, WQ)].rearrange(
                            "(a b) t -> a b t", b=HD)[:, HD - 1, :]
                        nc.sync.dma_start(qrow, ones16_bc[:, ts(tw, WQ)])
                        nc.sync.dma_start(
                            kTp[HD - 1 : HD, 2 * g, ts(tw, WQ)],
                            csf16_1p[:, ts(tw, WQ)])
                        nc.sync.dma_start(
                            kTp[P - 1 : P, 2 * g + 1, ts(tw, WQ)],
                            csf16_1p[:, ts(tw, WQ)])

                def scores_chunk(w, h, ex, kc):
                    """One k-chunk of scores + exp (+ causal mask) for (w,h)."""
                    h2, hg = h % 2, h // 2
                    hlo, hhi = HD * h2, HD * h2 + HD
                    d = kc - 4 * w
                    width = WQ if d < 0 else WQ - P * d
                    q0 = WQ * w + (WQ - width)
                    sc = psC.tile([P, WQ], F32, tag="sc")
                    mm(sc[:, 0:width],
                       kTp[:, h, ts(kc, P)],
                       qT[:, hg, q0 : q0 + width],
                       start=True, stop=True)
                    if use_kbias:
                        act(ex[:, kc, 0:width], sc[:, 0:width],
                            mybir.ActivationFunctionType.Exp,
                            bias=kb_sb[:, kc : kc + 1], scale=0.125)
                    else:
                        act(ex[:, kc, 0:width], sc[:, 0:width],
                            mybir.ActivationFunctionType.Exp,
                            scale=0.125)
                    if d >= 0:
                        nc.gpsimd.affine_select(
                            ex[:, kc, 0:P], ex[:, kc, 0:P],
                            pattern=[[1, P]],
                            compare_op=mybir.AluOpType.is_ge,
                            fill=0.0, base=0, channel_multiplier=-1)

                def attn_qt(w, h, ex, ql):
                    """attn @ v + normalization for one q-subtile of (w,h)."""
                    qt = 4 * w + ql
                    y_ps = psD.tile([P, HD + 1], F32, tag="y_ps")
                    for kc in range(qt + 1):
                        d = kc - 4 * w
                        toff = P * (qt - 4 * w) - (0 if d < 0 else P * d)
                        mm(y_ps[:], ex[:, kc, toff : toff + P],
                           vaug[:, kc, h, :],
                           start=(kc == 0), stop=(kc == qt))
                    rc = tmp.tile([P, 1], F32, tag="rc")
                    nc.vector.reciprocal(rc[:], y_ps[:, HD : HD + 1])
                    dve(nc.vector.tensor_scalar_mul,
                        yns[:, qt, ts(h, HD)], y_ps[:, 0:HD], rc[:])

                def epi_qt(w, ql):
                    """y transpose + output projection for one q-subtile."""
                    qt = 4 * w + ql
                    for g in range(2):
                        ty = psF.tile([P, P], F16, tag="oty")
                        nc.tensor.transpose(ty[:], yns[:, qt, ts(g, P)],
                                            ident[:])
                        if DUP_PE:
                            nc.tensor.transpose(ty[:], yns[:, qt, ts(g, P)],
                                                ident[:])
                        dve(nc.vector.tensor_copy, yT[:, g, ts(qt, P)], ty[:])
                    for half in range(2):
                        o_ps = psF.tile([P, 512], F32, tag="oty")
                        for cg in range(2):
                            mm(o_ps[:], yT[:, cg, ts(qt, P)],
                               w2sb[:, cg, ts(half, 512)],
                               start=(cg == 0), stop=(cg == 1))
                        o_sb = tmp.tile([P, 512], F32, tag="o_sb")
                        dve(nc.vector.tensor_copy, o_sb[:], o_ps[:])
                        nc.sync.dma_start(out_t[ts(qt, P), ts(half, 512)],
                                          o_sb[:])

                # Unified software pipeline, chunk-granular: while a
                # stream's exps drain on ACT, the PE queue carries the next
                # stream's scores matmuls; attn/epilogue work-slices of
                # previous streams are spread between scores chunks.
                if phases == 'a':
                    for tw in range(4):
                        aunit(tw)
                    continue
                from collections import deque
                slices = deque()

                def drain(n):
                    for _ in range(n):
                        if slices:
                            slices.popleft()()

                aunit(0)
                for w in range(NW):
                    nkc = 4 * w + 4
                    for h in range(HG):
                        ex = expool.tile([P, NT, WQ], F16, tag="ex",
                                         name=f"ex_{w}_{h}")
                        per = max(1, (nkc + 3) // 4)
                        for kc in range(nkc):
                            scores_chunk(w, h, ex, kc)
                            if kc % per == per - 1:
                                drain(1)
                        for ql in range(4):
                            slices.append(
                                lambda w=w, h=h, ex=ex, ql=ql:
                                attn_qt(w, h, ex, ql))
                        if h == HG - 1:
                            if w < NW - 1:
                                aunit(w + 1)
                            for ql in range(4):
                                slices.append(
                                    lambda w=w, ql=ql: epi_qt(w, ql))
                drain(len(slices))

    nc.compile()
    return nc


def _get_program(with_qkv_bias: bool, repeat: int = 1, phases: str = 'all',
                 use_kbias: bool = False, dup: str = ''):
    key = (bool(with_qkv_bias), repeat, phases, bool(use_kbias), dup)
    if key not in _PROGRAM_CACHE:
        _PROGRAM_CACHE[key] = _build_program(*key)
    return _PROGRAM_CACHE[key]


def _host_prep(x, cumulative_scores, padding_mask, token_index,
               c_attn_weight, c_attn_bias, c_proj_weight):
    """Host-side input prep: sharding + small index/trig math (O(B*T*HD))."""
    x = np.asarray(x, np.float32)
    cs = np.asarray(cumulative_scores, np.float32)
    pm = np.asarray(padding_mask, bool)
    ti = np.asarray(token_index, np.int64)
    Wa = np.asarray(c_attn_weight, np.float32)
    ba = np.asarray(c_attn_bias, np.float32)
    Wp = np.asarray(c_proj_weight, np.float32)

    # token histogram accumulates over ALL batches (faithful to reference)
    cnt = np.bincount(ti.ravel(), minlength=BLOCK).astype(np.float32)
    inv = np.float32(1.0) / (cnt + np.float32(1e-10))
    gathered = np.take_along_axis(np.broadcast_to(inv, (B, BLOCK)), ti, axis=1)
    pr = np.cumsum(gathered.astype(np.float32), axis=1, dtype=np.float32)

    inv_freq = 1.0 / ROPE_BASE ** (np.arange(0, HD, 2, dtype=np.float64) / HD)

    # permutation putting RoPE pair partners (d, d+32) in the same
    # 32-partition quadrant: [0:16, 32:48, 16:32, 48:64]
    perm = np.r_[0:16, 32:48, 16:32, 48:64]
    # cosT/sinsT tables in [head-dim(permuted) x 2 heads, t] layout; the
    # rotate_half sign is baked into sinsT.
    d_of_p = np.concatenate([perm, perm])          # [128]
    j_of_p = d_of_p % (HD // 2)
    sign_of_p = np.where(d_of_p < HD // 2, -1.0, 1.0)
    ang = pr[:, None, :].astype(np.float64) * inv_freq[j_of_p][None, :, None]
    cosT = np.cos(ang).astype(np.float16)                       # [B,128,T]
    sinsT = (sign_of_p[None, :, None] * np.sin(ang)).astype(np.float16)

    pmg = np.take_along_axis(pm, ti, axis=1)  # [B,T] gathered padding mask
    kb = np.where(pmg, np.float32(0.0), np.float32(NEG_BIG))
    ecs = np.exp(cs).astype(np.float32)
    csf16 = cs.astype(np.float16)
    ones16 = np.ones(T, np.float16)

    W3 = Wa.reshape(C, 3, H, HD)
    b3 = ba.reshape(3, H, HD)

    in_maps = []
    for core in range(N_CORES):
        b = core // (N_CORES // B)
        g0 = (core % (N_CORES // B)) * HG
        wq = W3[:, 0, g0 : g0 + HG, :][:, :, perm].reshape(C, CL)
        wk = W3[:, 1, g0 : g0 + HG, :][:, :, perm].reshape(C, CL)
        wv = W3[:, 2, g0 : g0 + HG, :].reshape(C, CL)
        wl = np.concatenate([wq, wk, wv], axis=1)
        ab = np.concatenate(
            [b3[0, g0 : g0 + HG, :][:, perm].reshape(CL),
             b3[1, g0 : g0 + HG, :][:, perm].reshape(CL),
             b3[2, g0 : g0 + HG, :].reshape(CL)])[None, :]
        w2 = Wp[g0 * HD : (g0 + HG) * HD, :].astype(np.float16)
        in_maps.append({
            "xT": np.ascontiguousarray(x[b].T).astype(np.float16),
            "wl": np.ascontiguousarray(wl).astype(np.float16),
            "w2": w2,
            "ab": np.ascontiguousarray(ab, np.float16),
            "cosT": cosT[b],
            "sinsT": sinsT[b],
            "ones16": ones16,
            "csf16": csf16[b],
            "ecs": ecs[b],
            "kb": kb[b],
        })
    return in_maps


def kernel(x, cumulative_scores, padding_mask, token_index,
           c_attn_weight, c_attn_bias, c_proj_weight, c_proj_bias):
    in_maps = _host_prep(x, cumulative_scores, padding_mask, token_index,
                         c_attn_weight, c_attn_bias, c_proj_weight)
    with_bias = bool(np.any(np.asarray(c_attn_bias) != 0))
    use_kbias = any((m["kb"] != 0).any() for m in in_maps)
    nc = _get_program(with_bias, 1, 'all', use_kbias)
    res = run_bass_kernel_spmd(nc, in_maps, list(range(N_CORES)))
    ncb = N_CORES // B
    bp = np.asarray(c_proj_bias, np.float32)
    out = np.empty((B, T, C), np.float32)
    for b in range(B):
        acc = res.results[b * ncb]["out"].astype(np.float32)
        for j in range(1, ncb):
            acc = acc + res.results[b * ncb + j]["out"]
        out[b] = acc + bp[None, :]
    return out

